# revision 8
# baseline (speedup 1.0000x reference)
"""External Attention (nn_External_Attention) on 8 TRN2 NeuronCores.

kernel(x, Wk, Wv) -> x + Wv @ l1norm_M(softmax_N(Wk @ x))
  x  [16, 512, 4096] f32,  Wk [256, 512] f32,  Wv [512, 256] f32

Sharding: data-parallel over batch B=16 -> 2 batches per core across 8 cores.

v2 design (vs the 224us f32r baseline):
  - x is shipped bf16 (halves input DMA), y is produced bf16 on device and
    upcast f32 on host after the gather (halves output DMA).
  - phase A: logits = WkT.T @ x in bf16 (full PE rate, FWL weight loads),
    E = exp(logits - 3) quantized to fp8e4 by the ACT exp (softmax is
    shift-invariant; the -3 bias keeps E inside e4m3 range), row sums
    accumulated f32 by the same ACT op.
  - stats: rr = 1/rowsum; rr8 = fp8(rr*4096); WVP = fp8(wvT*4096 * rr)
    (the 4096 pre-scale rides in the host-side wv upload; it cancels against
    the 4096 in the cs matmul, keeping the mm2 output at unit scale while
    both fp8 operand sets sit in healthy e4m3/e5m2 range).
  - cs = rr8.T @ E per 512-col tile: ONE DoubleRow fp8 matmul (K=256),
    bc = broadcast(1/cs) (ACT reciprocal + GPSIMD partition_broadcast).
  - E' = E * bc -> fp8e5 (GPSIMD), mm2: po = WVP.T @ E' as ONE DoubleRow
    fp8 matmul per (co, j) -- half the PE cycles of the bf16 version.
  - residual: batch 0 adds x on DVE during PSUM evacuation (slack there);
    batch 1 (the pipeline tail) accumulates identity @ x into the same PSUM
    on the PE and evacuates with cheap single-src copies split DVE/ACT,
    keeping the tail at PE speed.

Rel err vs the fp32 reference ~2.4e-3 (dominated by the bf16 x/y
quantization; gate is 2e-2).
"""
from contextlib import ExitStack

import numpy as np
import ml_dtypes

import concourse.bacc as bacc
import concourse.mybir as mybir
import concourse.tile as tile
from concourse.bass_utils import run_bass_kernel_spmd

F32 = mybir.dt.float32
BF16 = mybir.dt.bfloat16
FP8E4 = mybir.dt.float8e4
FP8E5 = mybir.dt.float8e5
AF = mybir.ActivationFunctionType
ALU = mybir.AluOpType
AX = mybir.AxisListType
DR = mybir.MatmulPerfMode.DoubleRow

B, C, M, N = 16, 512, 256, 4096
NCORES = 8
BPC = B // NCORES
NT = 512
KC = C // 128   # 4
KM = M // 128   # 2
NJ = N // NT    # 8
XH = 1024
NH = N // XH
JH = XH // NT
EXP_BIAS = -3.0
S = 4096.0      # rr scale; folded into the host-side wv upload


def _act_reciprocal(nc, out_ap, in_ap):
    """InstActivation(func=Reciprocal) emitted directly (the helper bans it
    for precision; HW-measured max rel err 1.2e-5 -- fine for the colsum
    normalizer)."""
    eng = nc.scalar
    inputs = [eng.lower_ap(in_ap),
              mybir.ImmediateValue(dtype=mybir.dt.float32, value=0.0),
              mybir.ImmediateValue(dtype=mybir.dt.float32, value=1.0),
              mybir.ImmediateValue(dtype=mybir.dt.float32, value=0.0)]
    return eng.add_instruction(
        mybir.InstActivation(
            name=nc.get_next_instruction_name(),
            func=AF.Reciprocal,
            ins=inputs,
            outs=[eng.lower_ap(out_ap)],
        )
    )


def _build(nc):
    x_d = nc.dram_tensor("x", [BPC, C, N], BF16, kind="ExternalInput").ap()
    wkT_d = nc.dram_tensor("wkT", [C, M], BF16, kind="ExternalInput").ap()
    wvT_d = nc.dram_tensor("wvT", [M, C], F32, kind="ExternalInput").ap()
    id_d = nc.dram_tensor("ident", [128, 128], BF16, kind="ExternalInput").ap()
    y_d = nc.dram_tensor("y", [BPC, C, N], BF16, kind="ExternalOutput").ap()

    with tile.TileContext(nc) as tc, ExitStack() as ctx:
        wpool = ctx.enter_context(tc.tile_pool(name="w", bufs=1))
        xpool = ctx.enter_context(tc.tile_pool(name="xp", bufs=33))
        epool = ctx.enter_context(tc.tile_pool(name="ep", bufs=2))
        eppool = ctx.enter_context(tc.tile_pool(name="epp", bufs=10))
        spool = ctx.enter_context(tc.tile_pool(name="sp", bufs=4))
        wvppool = ctx.enter_context(tc.tile_pool(name="wvp", bufs=2))
        ypool = ctx.enter_context(tc.tile_pool(name="yp", bufs=10))
        bcpool = ctx.enter_context(tc.tile_pool(name="bcp", bufs=6))
        ps_l = ctx.enter_context(tc.tile_pool(name="ps_l", bufs=2, space="PSUM"))
        ps_cs = ctx.enter_context(tc.tile_pool(name="ps_cs", bufs=2, space="PSUM"))
        ps_o = ctx.enter_context(tc.tile_pool(name="ps_o", bufs=4, space="PSUM"))

        wk_sb = []
        for kc in range(KC):
            t = wpool.tile([128, M], BF16, tag=f"wk{kc}", name=f"wk{kc}")
            nc.sync.dma_start(t[:], wkT_d[kc * 128:(kc + 1) * 128, :])
            wk_sb.append(t)
        wv_sb = []
        for km in range(KM):
            t = wpool.tile([128, C], F32, tag=f"wv{km}", name=f"wv{km}")
            nc.sync.dma_start(t[:], wvT_d[km * 128:(km + 1) * 128, :])
            wv_sb.append(t)
        id_sb = wpool.tile([128, 128], BF16, tag="id", name="id")
        nc.sync.dma_start(id_sb[:], id_d[:, :])
        ebias = wpool.tile([128, 1], F32, tag="ebias", name="ebias")
        nc.gpsimd.memset(ebias[:], EXP_BIAS)

        X, E, RSP, RR8, WVP, BC = {}, {}, {}, {}, {}, {}
        EPT = {}

        def load_x(b):
            x_sb = [[None] * KC for _ in range(NH)]
            for h in range(NH):
                for kc in range(KC):
                    t = xpool.tile([128, XH], BF16, tag="x", name=f"x{b}_{h}_{kc}")
                    nc.sync.dma_start(
                        t[:], x_d[b, kc * 128:(kc + 1) * 128, h * XH:(h + 1) * XH])
                    x_sb[h][kc] = t
            X[b] = x_sb

        def xs(b, kc, j):
            h, jj = j // JH, j % JH
            return X[b][h][kc][:, jj * NT:(jj + 1) * NT]

        def init_A(b):
            E[b] = epool.tile([128, KM, N], FP8E4, tag="e", name=f"e{b}")
            RSP[b] = [spool.tile([128, NJ], F32, tag="rsp", name=f"rsp{b}_{km}")
                      for km in range(KM)]

        def emit_A(b, j):
            for km in range(KM):
                pl = ps_l.tile([128, NT], F32, tag="pl", name=f"pl{b}_{j}_{km}")
                for kc in range(KC):
                    nc.tensor.matmul(pl[:], wk_sb[kc][:, km * 128:(km + 1) * 128],
                                     xs(b, kc, j),
                                     start=(kc == 0), stop=(kc == KC - 1))
                nc.scalar.activation(E[b][:, km, j * NT:(j + 1) * NT], pl[:],
                                     AF.Exp, bias=ebias[:],
                                     accum_out=RSP[b][km][:, j:j + 1])

        def emit_stats(b):
            # rr8 layout [128, 2, 16]: the DoubleRow cs lhsT slice [:, :, 0:1]
            # needs the ko dim at a 16-byte step
            rr8 = spool.tile([128, KM, 16], FP8E4, tag="rr8", name=f"rr8{b}")
            wvp = wvppool.tile([128, KM, C], FP8E4, tag="wvp", name=f"wvp{b}")
            for km in range(KM):
                rs = spool.tile([128, 1], F32, tag="rs", name=f"rs{b}_{km}")
                nc.vector.tensor_reduce(rs[:], RSP[b][km][:], axis=AX.X, op=ALU.add)
                rr = spool.tile([128, 1], F32, tag="rr", name=f"rr{b}_{km}")
                nc.vector.reciprocal(rr[:], rs[:])
                nc.vector.tensor_scalar_mul(rr8[:, km, 0:1], rr[:], S)
                nc.vector.tensor_scalar_mul(wvp[:, km, :], wv_sb[km][:], rr[:])
            RR8[b], WVP[b] = rr8, wvp

        def emit_cs(b, j):
            cs = ps_cs.tile([1, NT], F32, tag="cs", name=f"cs{b}_{j}")
            nc.tensor.matmul(cs[:], RR8[b][:, :, 0:1],
                             E[b][:, :, j * NT:(j + 1) * NT],
                             start=True, stop=True, perf_mode=DR)
            rcs = bcpool.tile([1, NT], F32, tag="rcs", name=f"rcs{b}_{j}")
            _act_reciprocal(nc, rcs[:], cs[:])
            bc = bcpool.tile([128, NT], F32, tag="bc", name=f"bc{b}_{j}")
            nc.gpsimd.partition_broadcast(bc[:], rcs[:])
            BC[(b, j)] = bc

        def emit_ep(b, j):
            bc = BC.pop((b, j))
            ep = eppool.tile([128, KM, NT], FP8E5, tag="epp", name=f"epp{b}_{j}")
            for t in range(KM):
                nc.gpsimd.tensor_tensor(ep[:, t, :],
                                        E[b][:, t, j * NT:(j + 1) * NT],
                                        bc[:], op=ALU.mult)
            EPT[(b, j)] = ep

        def emit_mm2(b, j, co, pe_residual, evac_idx):
            ep = EPT[(b, j)]
            po = ps_o.tile([128, NT], F32, tag="po", name=f"po{b}_{j}_{co}")
            nc.tensor.matmul(po[:], WVP[b][:, :, co * 128:(co + 1) * 128], ep[:],
                             start=True, stop=not pe_residual, perf_mode=DR)
            yt = ypool.tile([128, NT], BF16, tag="y", name=f"y{b}_{j}_{co}")
            if pe_residual:
                nc.tensor.matmul(po[:], id_sb[:], xs(b, co, j),
                                 start=False, stop=True)
                if evac_idx % 2 == 0:
                    nc.vector.tensor_copy(yt[:], po[:])
                else:
                    nc.scalar.copy(yt[:], po[:])
            else:
                nc.vector.tensor_tensor(yt[:], po[:], xs(b, co, j), op=ALU.add)
            nc.sync.dma_start(
                y_d[b, co * 128:(co + 1) * 128, j * NT:(j + 1) * NT], yt[:])

        # ---- emission schedule (engine streams execute in this order) ----
        load_x(0)
        load_x(1)
        for b in range(BPC):
            init_A(b)
            for j in range(NJ):
                emit_A(b, j)
            emit_stats(b)
        evac = [0]
        for b in range(BPC):
            for j in range(NJ):
                emit_cs(b, j)
            pe_res = (b == BPC - 1)
            if not pe_res:
                # co-outer: each WVP column block is loaded once per 8 j's
                for j in range(NJ):
                    emit_ep(b, j)
                for co in range(KC):
                    for j in range(NJ):
                        emit_mm2(b, j, co, False, 0)
                for j in range(NJ):
                    EPT.pop((b, j))
            else:
                for j in range(NJ):
                    emit_ep(b, j)
                    for co in range(KC):
                        emit_mm2(b, j, co, True, evac[0])
                        evac[0] += 1
                    EPT.pop((b, j))
    return nc


_CACHE = {}


def _get_program():
    if "nc" not in _CACHE:
        nc = bacc.Bacc("TRN2", target_bir_lowering=False, debug=False,
                       enable_asserts=True)
        _build(nc)
        nc.compile()
        _CACHE["nc"] = nc
    return _CACHE["nc"]


def _prep_inputs(x, Wk, Wv):
    xb = np.ascontiguousarray(np.asarray(x, dtype=np.float32)).astype(
        ml_dtypes.bfloat16)
    wkT = np.ascontiguousarray(
        np.asarray(Wk, dtype=np.float32).T).astype(ml_dtypes.bfloat16)
    wvT = np.ascontiguousarray(np.asarray(Wv, dtype=np.float32).T * np.float32(S))
    ident = np.eye(128, dtype=np.float32).astype(ml_dtypes.bfloat16)
    return xb, wkT, wvT, ident


def kernel(x, Wk, Wv):
    xb, wkT, wvT, ident = _prep_inputs(x, Wk, Wv)
    nc = _get_program()
    in_maps = [{"x": xb[i * BPC:(i + 1) * BPC], "wkT": wkT, "wvT": wvT,
                "ident": ident}
               for i in range(NCORES)]
    res = run_bass_kernel_spmd(nc, in_maps, list(range(NCORES)))
    y = np.concatenate([res.results[i]["y"] for i in range(NCORES)], axis=0)
    return np.ascontiguousarray(y.astype(np.float32))
